# revision 31
# baseline (speedup 1.0000x reference)
"""Cross-modal triplet loss (hard mining) on 8 Trainium2 NeuronCores.

Math: for row i with modality m_i and target t_i over n=16384 samples
(first half modality 0, second half modality 1):
    d2(i,j) = ||x_i||^2 + ||x_j||^2 - 2 x_i.x_j
    dist_ap_i = max over cross-modal same-target j   of sqrt(clip(d2))
    dist_an_i = min over cross-modal other-target j  of sqrt(clip(d2))
    loss = mean(relu(dist_ap - dist_an + 0.3));  correct = sum(dist_an >= dist_ap)

Strategy (v2 — tensor-bound fix):
 - Only cross-modal pairs matter -> 8 cores x 2048 rows each vs the 8192
   columns of the opposite half.  PSUM holds v' = 2g - sq_j per row tile.
 - Host sorts each half by target id and rotates columns per core so row
   tile rt's positives live in local columns [128*rt, 128*rt + W).
 - PE: per (group, row-tile) the 4 nsq matmuls (ones stationary) run
   first, then the 4 main matmuls (lhsT stationary): 2 weight swaps per
   2048 columns instead of 8.
 - Reduction work is split across engines so the PE never waits:
     * window chunks (1024-wide, containing the positive band): DVE
       TENSOR_MASK_REDUCE with per-row inverted ranges -> neg partial;
       ACT negates the 512 window for the pos-path masked max.
     * plain chunks: some reduced directly by DVE from PSUM
       (tensor_reduce max), the rest copied PSUM->SBUF fp16 by ACT and
       max-folded by GpSimd into a per-tile accumulator.
 - Final per-tile combine (DVE) merges all partials; host adds sq_i,
   takes sqrt, computes loss/correct.
"""

import numpy as np
import ml_dtypes

N_TOTAL = 16384
HALF = 8192
FEAT = 128
N_CORES = 8
ROWS = 2048          # rows per core
N_RT = 16            # row tiles per core (128 rows each)
CH = 1024            # psum chunk width (2 banks)
N_CH = HALF // CH    # 8 chunks
GCOL = 2048          # rhs DMA group width
N_G = 4
W = 512              # positive-band window width
PAD = 192            # rotation pad; requires max target multiplicity <= PAD
MARGIN = 0.3
PART_STRIDE = 10     # partial columns reserved per row tile

BF16 = ml_dtypes.bfloat16
FP16 = np.float16


def _bf16_split3(x):
    """Split fp32 array into 3 bf16 levels summing to x (to ~2^-27 rel)."""
    h = x.astype(BF16)
    r1 = x - h.astype(np.float32)
    m = r1.astype(BF16)
    r2 = r1 - m.astype(np.float32)
    l = r2.astype(BF16)
    return np.stack([np.asarray(h), np.asarray(m), np.asarray(l)], axis=0)


def _plan():
    """Static per-row-tile plan: window chunks, plain chunk assignment,
    wb spans, partial-column layout. Data independent."""
    plan = []
    ncol_neg = 0
    for rt in range(N_RT):
        wlo, whi = 128 * rt, 128 * rt + W
        wcs = sorted({wlo // CH, (whi - 1) // CH})
        plain = [c for c in range(N_CH) if c not in wcs]
        # wb spans: (chunk, lo_loc, hi_loc, wb_off)
        spans = []
        for c in wcs:
            lo = max(wlo, c * CH)
            hi = min(whi, (c + 1) * CH)
            if lo < hi:
                spans.append((c, lo - c * CH, hi - c * CH, lo - wlo))
        negcols = {c: ncol_neg + i for i, c in enumerate(wcs)}
        ncol_neg += len(wcs)
        plan.append(dict(wcs=wcs, plain=plain, spans=spans,
                         negcols=negcols))
    return plan, ncol_neg


_PLAN, _NCOL_NEG = _plan()

_MODULES = {}


def _build_module_fast():
    import concourse.bacc as bacc
    import concourse.tile as tile
    import concourse.mybir as mybir
    from concourse.dve_ops import TENSOR_MASK_REDUCE

    dt = mybir.dt
    plan = _PLAN

    nc = bacc.Bacc("TRN2", target_bir_lowering=False, debug=False,
                   enable_asserts=False, num_devices=1)

    d_lhsT = nc.dram_tensor("lhsT", [FEAT, ROWS], dt.bfloat16,
                            kind="ExternalInput").ap()
    d_rhs = nc.dram_tensor("rhs", [FEAT, HALF], dt.bfloat16,
                           kind="ExternalInput").ap()
    d_sqf = nc.dram_tensor("sqf", [128, HALF], dt.float32,
                           kind="ExternalInput").ap()
    d_nsq = nc.dram_tensor("nsq", [3, HALF], dt.bfloat16,
                           kind="ExternalInput").ap()
    d_ones = nc.dram_tensor("ones", [3, FEAT], dt.bfloat16,
                            kind="ExternalInput").ap()
    d_negc0 = nc.dram_tensor("negc0", [128, _NCOL_NEG], dt.float32,
                             kind="ExternalInput").ap()
    d_negc3 = nc.dram_tensor("negc3", [128, _NCOL_NEG], dt.float32,
                             kind="ExternalInput").ap()
    d_maxs = nc.dram_tensor("maxs", [128, N_RT], dt.float32,
                            kind="ExternalInput").ap()
    d_maxe = nc.dram_tensor("maxe", [128, N_RT], dt.float32,
                            kind="ExternalInput").ap()
    d_out = nc.dram_tensor("out", [128, 2 * N_RT], dt.float32,
                           kind="ExternalOutput").ap()

    with tile.TileContext(nc) as tc:
        with tc.tile_pool(name="const", bufs=1) as cpool, \
             tc.tile_pool(name="psum", bufs=4, space="PSUM") as ppool, \
             tc.tile_pool(name="scr", bufs=3) as spool, \
             tc.tile_pool(name="wb", bufs=4) as wpool:

            t_lhsT = cpool.tile([FEAT, ROWS], dt.bfloat16)
            t_sqf = cpool.tile([128, HALF], dt.float32)
            t_nsq = cpool.tile([3, HALF], dt.bfloat16)
            t_ones = cpool.tile([3, FEAT], dt.bfloat16)
            t_negc0 = cpool.tile([128, _NCOL_NEG], dt.float32)
            t_negc3 = cpool.tile([128, _NCOL_NEG], dt.float32)
            t_maxs = cpool.tile([128, N_RT], dt.float32)
            t_maxe = cpool.tile([128, N_RT], dt.float32)
            t_out = cpool.tile([128, 2 * N_RT], dt.float32)
            t_part = cpool.tile([128, PART_STRIDE * N_RT], dt.float32)
            t_chain = cpool.tile([128, N_RT], dt.float32)

            rhs_t = []
            for g in range(N_G):
                t = cpool.tile([FEAT, GCOL], dt.bfloat16, tag=f"rhs{g}",
                               name=f"rhs{g}")
                rhs_t.append(t)
            eng = [nc.sync, nc.scalar, nc.gpsimd, nc.sync]
            nc.sync.dma_start(rhs_t[0][:], d_rhs[:, 0:GCOL])
            nc.scalar.dma_start(t_lhsT[:], d_lhsT)
            nc.scalar.dma_start(t_sqf[:, :HALF // 2], d_sqf[:, :HALF // 2])
            nc.gpsimd.dma_start(t_sqf[:, HALF // 2:], d_sqf[:, HALF // 2:])
            nc.gpsimd.dma_start(t_nsq[:], d_nsq)
            nc.gpsimd.dma_start(t_ones[:], d_ones)
            nc.gpsimd.dma_start(t_negc0[:], d_negc0)
            nc.gpsimd.dma_start(t_negc3[:], d_negc3)
            nc.gpsimd.dma_start(t_maxs[:], d_maxs)
            nc.gpsimd.dma_start(t_maxe[:], d_maxe)
            for g in range(1, N_G):
                eng[g].dma_start(rhs_t[g][:], d_rhs[:, g * GCOL:(g + 1) * GCOL])

            # warm up all PSUM banks with a dummy start=True matmul so the
            # later start=False accumulation onto ACT-preloaded content works
            for w in range(4):
                wt = ppool.tile([128, CH], dt.float32, tag="ps",
                                name=f"warm{w}")
                for k in range(2):
                    nc.tensor.matmul(
                        wt[:, 512 * k:512 * k + 512],
                        t_lhsT[:, 0:128], t_lhsT[:, 0:512],
                        start=True, stop=True, skip_group_check=True)

            wb_tiles = {}

            for g in range(N_G):
                for rt in range(N_RT):
                    p = plan[rt]
                    ps = [ppool.tile([128, CH], dt.float32, tag="ps",
                                     name=f"ps{g}_{rt}_{c}")
                          for c in range(2)]
                    # -sq_j into PSUM: chunks 6,7 via nsq matmul (PE has
                    # headroom), others via ACT preload; matmuls add 2g
                    for cloc in range(2):
                        gc = 2 * g + cloc
                        if gc >= 6:
                            for k in range(2):
                                nc.tensor.matmul(
                                    ps[cloc][:, 512 * k:512 * k + 512],
                                    t_ones[:],
                                    t_nsq[:, gc * CH + 512 * k:
                                          gc * CH + 512 * k + 512],
                                    start=True, stop=False)
                        else:
                            nc.scalar.copy(ps[cloc][:],
                                           t_sqf[:, gc * CH:(gc + 1) * CH])
                    for cloc in range(2):
                        for k in range(2):
                            sl = slice(512 * k, 512 * k + 512)
                            nc.tensor.matmul(
                                ps[cloc][:, sl],
                                t_lhsT[:, 128 * rt:128 * rt + 128],
                                rhs_t[g][:, cloc * CH + 512 * k:
                                          cloc * CH + 512 * k + 512],
                                start=False, stop=True,
                                skip_group_check=True)

                    # consumers: window chunks first, then plain chunks
                    plain_here = []
                    for cloc in range(2):
                        gc = 2 * g + cloc
                        t = ps[cloc]
                        if gc not in p["wcs"]:
                            plain_here.append((gc, t))
                            continue
                        if True:
                            wi = p["wcs"].index(gc)
                            col = p["negcols"][gc]
                            last = wi == len(p["wcs"]) - 1
                            seed = (-3.0e38 if wi == 0
                                    else t_chain[:, rt:rt + 1])
                            accout = (t_part[:, PART_STRIDE * rt:
                                             PART_STRIDE * rt + 1]
                                      if last else t_chain[:, rt:rt + 1])
                            scr = spool.tile([128, CH], dt.float32,
                                             tag="scr", name="scrn")
                            nc.vector._custom_dve(
                                TENSOR_MASK_REDUCE, out=scr[:], in0=t[:],
                                in1=t_negc3[:, col:col + 1],
                                s0=t_negc0[:, col:col + 1],
                                s1=seed, imm2=1.0, accum_out=accout)
                            # wb parts for pos path
                            for (c_, lo, hi, off) in p["spans"]:
                                if c_ != gc:
                                    continue
                                if rt not in wb_tiles:
                                    wb_tiles[rt] = wpool.tile(
                                        [128, W], dt.float32, tag="wb",
                                        name=f"wb{rt}")
                                nc.scalar.mul(
                                    wb_tiles[rt][:, off:off + (hi - lo)],
                                    t[:, lo:hi], -1.0)
                            if last:
                                scr2 = spool.tile([128, CH], dt.float32,
                                                  tag="scr", name="scrp")
                                nc.vector._custom_dve(
                                    TENSOR_MASK_REDUCE, out=scr2[:, :W],
                                    in0=wb_tiles[rt][:],
                                    in1=t_maxe[:, rt:rt + 1],
                                    s0=t_maxs[:, rt:rt + 1],
                                    s1=-3.0e38, imm2=1.0,
                                    accum_out=t_out[:, 2 * rt:2 * rt + 1])
                    # plain chunks: DVE native max-reduce straight from PSUM
                    for gc, t in plain_here:
                        col = PART_STRIDE * rt + 1 + p["plain"].index(gc)
                        nc.vector.tensor_reduce(
                            out=t_part[:, col:col + 1], in_=t[:],
                            axis=mybir.AxisListType.X,
                            op=mybir.AluOpType.max)

            # per-tile combine
            for rt in range(N_RT):
                width = 1 + len(plan[rt]["plain"])
                nc.vector.tensor_reduce(
                    out=t_out[:, 2 * rt + 1:2 * rt + 2],
                    in_=t_part[:, PART_STRIDE * rt:PART_STRIDE * rt + width],
                    axis=mybir.AxisListType.X, op=mybir.AluOpType.max)

            nc.sync.dma_start(d_out, t_out[:])

    nc.compile()
    from concourse.bass_interp import get_hw_module
    nc.m = get_hw_module(nc.m)
    return nc


# ---------------------------------------------------------------- fallback
def _segments_fallback():
    return [[(g, 0, GCOL) for g in range(N_G)] for _ in range(N_RT)]


def _build_module_fallback():
    """Original v1 structure: full masked reduce over every 2048 group.
    Used only if the fast-path layout assumptions fail for the data."""
    import concourse.bacc as bacc
    import concourse.tile as tile
    import concourse.mybir as mybir
    from concourse.dve_ops import TENSOR_MASK_REDUCE

    dt = mybir.dt
    segs = _segments_fallback()
    nseg = sum(len(s) for s in segs)
    segcols = {}
    c = 0
    for rt in range(N_RT):
        for si in range(len(segs[rt])):
            segcols[(rt, si)] = c
            c += 1

    nc = bacc.Bacc("TRN2", target_bir_lowering=False, debug=False,
                   enable_asserts=False, num_devices=1)

    d_lhsT = nc.dram_tensor("lhsT", [FEAT, ROWS], dt.bfloat16,
                            kind="ExternalInput").ap()
    d_rhs = nc.dram_tensor("rhs", [FEAT, HALF], dt.bfloat16,
                           kind="ExternalInput").ap()
    d_nsq = nc.dram_tensor("nsq", [3, HALF], dt.bfloat16,
                           kind="ExternalInput").ap()
    d_ones = nc.dram_tensor("ones", [3, FEAT], dt.bfloat16,
                            kind="ExternalInput").ap()
    d_minc0 = nc.dram_tensor("minc0", [128, N_RT * N_G], dt.float32,
                             kind="ExternalInput").ap()
    d_minc3 = nc.dram_tensor("minc3", [128, N_RT * N_G], dt.float32,
                             kind="ExternalInput").ap()
    d_maxs = nc.dram_tensor("maxs", [128, nseg], dt.float32,
                            kind="ExternalInput").ap()
    d_maxe = nc.dram_tensor("maxe", [128, nseg], dt.float32,
                            kind="ExternalInput").ap()
    d_out = nc.dram_tensor("out", [128, 2 * N_RT], dt.float32,
                           kind="ExternalOutput").ap()

    with tile.TileContext(nc) as tc:
        with tc.tile_pool(name="const", bufs=1) as cpool, \
             tc.tile_pool(name="psum", bufs=2, space="PSUM") as ppool, \
             tc.tile_pool(name="scr", bufs=3) as spool, \
             tc.tile_pool(name="wb", bufs=3) as wpool:

            t_lhsT = cpool.tile([FEAT, ROWS], dt.bfloat16)
            t_nsq = cpool.tile([3, HALF], dt.bfloat16)
            t_ones = cpool.tile([3, FEAT], dt.bfloat16)
            t_minc0 = cpool.tile([128, N_RT * N_G], dt.float32)
            t_minc3 = cpool.tile([128, N_RT * N_G], dt.float32)
            t_maxs = cpool.tile([128, nseg], dt.float32)
            t_maxe = cpool.tile([128, nseg], dt.float32)
            t_out = cpool.tile([128, 2 * N_RT], dt.float32)
            t_accn = cpool.tile([128, N_RT * N_G], dt.float32)
            t_acca = cpool.tile([128, nseg], dt.float32)

            rhs_t = []
            for g in range(N_G):
                t = cpool.tile([FEAT, GCOL], dt.bfloat16, tag=f"rhs{g}",
                               name=f"rhs{g}")
                rhs_t.append(t)
            eng = [nc.sync, nc.scalar, nc.gpsimd, nc.sync]
            nc.sync.dma_start(rhs_t[0][:], d_rhs[:, 0:GCOL])
            nc.scalar.dma_start(t_lhsT[:], d_lhsT)
            nc.gpsimd.dma_start(t_nsq[:], d_nsq)
            nc.gpsimd.dma_start(t_ones[:], d_ones)
            nc.gpsimd.dma_start(t_minc0[:], d_minc0)
            nc.gpsimd.dma_start(t_minc3[:], d_minc3)
            nc.gpsimd.dma_start(t_maxs[:], d_maxs)
            nc.gpsimd.dma_start(t_maxe[:], d_maxe)
            for g in range(1, N_G):
                eng[g].dma_start(rhs_t[g][:], d_rhs[:, g * GCOL:(g + 1) * GCOL])

            for g in range(N_G):
                for rt in range(N_RT):
                    ps = ppool.tile([128, GCOL], dt.float32, tag="ps",
                                    name="ps")
                    for k in range(GCOL // 512):
                        sl = slice(512 * k, 512 * k + 512)
                        nc.tensor.matmul(
                            ps[:, sl],
                            t_lhsT[:, 128 * rt:128 * rt + 128],
                            rhs_t[g][:, sl], start=True, stop=False)
                        nc.tensor.matmul(
                            ps[:, sl], t_ones[:],
                            t_nsq[:, g * GCOL + 512 * k:
                                  g * GCOL + 512 * k + 512],
                            start=False, stop=True)

                    col = rt * N_G + g
                    seed = -3.0e38 if g == 0 else t_accn[:, col - 1:col]
                    accout = (t_out[:, 2 * rt + 1:2 * rt + 2]
                              if g == N_G - 1 else t_accn[:, col:col + 1])
                    scr = spool.tile([128, GCOL], dt.float32, tag="scr",
                                     name="scr")
                    nc.vector._custom_dve(
                        TENSOR_MASK_REDUCE, out=scr[:], in0=ps[:],
                        in1=t_minc3[:, col:col + 1],
                        s0=t_minc0[:, col:col + 1],
                        s1=seed, imm2=1.0, accum_out=accout)

                    for si, (sg, lo, hi) in enumerate(segs[rt]):
                        if sg != g:
                            continue
                        L = hi - lo
                        scol = segcols[(rt, si)]
                        wb = wpool.tile([128, GCOL], dt.float32,
                                        tag="wb", name="wb")
                        nc.scalar.mul(wb[:, :L], ps[:, lo:hi], -1.0)
                        seed_a = (-3.0e38 if si == 0
                                  else t_acca[:, scol - 1:scol])
                        accout_a = (t_out[:, 2 * rt:2 * rt + 1]
                                    if si == len(segs[rt]) - 1
                                    else t_acca[:, scol:scol + 1])
                        scr2 = spool.tile([128, GCOL], dt.float32,
                                          tag="scr", name="scr2")
                        nc.vector._custom_dve(
                            TENSOR_MASK_REDUCE, out=scr2[:, :L],
                            in0=wb[:, :L],
                            in1=t_maxe[:, scol:scol + 1],
                            s0=t_maxs[:, scol:scol + 1],
                            s1=seed_a, imm2=1.0, accum_out=accout_a)

            nc.sync.dma_start(d_out, t_out[:])

    nc.compile()
    from concourse.bass_interp import get_hw_module
    nc.m = get_hw_module(nc.m)
    return nc


def _host_prep(inputs, targets):
    x = np.ascontiguousarray(np.asarray(inputs), dtype=np.float32)
    t = np.asarray(targets)
    sq = (x.astype(np.float64) ** 2).sum(axis=1)   # host-side exact
    sq32 = (x * x).sum(axis=1, dtype=np.float32)   # device-side value

    halves = [np.arange(0, HALF), np.arange(HALF, N_TOTAL)]
    order = []
    for h in range(2):
        idx = halves[h]
        perm = np.argsort(t[idx], kind="stable")
        order.append(idx[perm])

    fast = True
    core_rows = []
    core_info = []
    for c in range(N_CORES):
        cp = c % 4
        rows = order[0 if c < 4 else 1][cp * ROWS:(cp + 1) * ROWS]
        cols_sorted = order[1 if c < 4 else 0]
        tcols = t[cols_sorted]
        trows = t[rows]
        s_g = np.searchsorted(tcols, trows, side="left")
        e_g = np.searchsorted(tcols, trows, side="right")
        r = cp * ROWS - PAD
        l_s = (s_g - r) % HALF
        l_e = l_s + (e_g - s_g)
        rt_idx = np.arange(ROWS) // 128
        ok = (np.all(e_g > s_g)
              and np.all(l_s >= 128 * rt_idx)
              and np.all(l_e <= 128 * rt_idx + W))
        fast = fast and bool(ok)
        core_rows.append((rows, e_g - s_g))
        core_info.append((rows, cols_sorted, r, s_g, e_g))

    in_maps = []
    ones = np.ones((3, FEAT), dtype=BF16)
    for c in range(N_CORES):
        rows, cols_sorted, r, s_g, e_g = core_info[c]
        if fast:
            cols_rot = np.roll(cols_sorted, -r)
            l_s = (s_g - r) % HALF
        else:
            cols_rot = cols_sorted
            l_s = s_g
        l_e = l_s + (e_g - s_g)
        lhsT = np.ascontiguousarray((2.0 * x[rows]).T.astype(BF16))
        rhs = np.ascontiguousarray(x[cols_rot].T.astype(BF16))
        ls2 = l_s.reshape(N_RT, 128)
        le2 = l_e.reshape(N_RT, 128)

        if fast:
            negc0 = np.zeros((128, _NCOL_NEG), dtype=np.float32)
            negc3 = np.zeros((128, _NCOL_NEG), dtype=np.float32)
            maxs = np.zeros((128, N_RT), dtype=np.float32)
            maxe = np.zeros((128, N_RT), dtype=np.float32)
            for rt in range(N_RT):
                p = _PLAN[rt]
                for wc in p["wcs"]:
                    col = p["negcols"][wc]
                    ls_loc = np.clip(ls2[rt] - wc * CH, 0, CH)
                    le_loc = np.clip(le2[rt] - wc * CH, 0, CH)
                    empty = ls_loc >= le_loc
                    c0 = le_loc.astype(np.float32)
                    c3 = ls_loc.astype(np.float32)
                    c0[empty] = 0.0
                    c3[empty] = float(CH)
                    negc0[:, col] = c0
                    negc3[:, col] = c3
                maxs[:, rt] = ls2[rt] - 128 * rt
                maxe[:, rt] = le2[rt] - 128 * rt
            sqf = np.ascontiguousarray(np.broadcast_to(
                (-sq32[cols_rot]).astype(np.float32), (128, HALF)))
            nsq = np.ascontiguousarray(_bf16_split3(-sq32[cols_rot]))
            in_maps.append({
                "lhsT": lhsT, "rhs": rhs, "sqf": sqf, "nsq": nsq,
                "ones": ones,
                "negc0": negc0, "negc3": negc3, "maxs": maxs, "maxe": maxe,
            })
        else:
            nsq = np.ascontiguousarray(_bf16_split3(-sq32[cols_rot]))
            segs = _segments_fallback()
            nseg = sum(len(s) for s in segs)
            segcols = {}
            cc = 0
            for rt in range(N_RT):
                for si in range(len(segs[rt])):
                    segcols[(rt, si)] = cc
                    cc += 1
            minc0 = np.zeros((128, N_RT * N_G), dtype=np.float32)
            minc3 = np.zeros((128, N_RT * N_G), dtype=np.float32)
            maxs = np.zeros((128, nseg), dtype=np.float32)
            maxe = np.zeros((128, nseg), dtype=np.float32)
            for rt in range(N_RT):
                for g in range(N_G):
                    sg = np.clip(ls2[rt] - g * GCOL, 0, GCOL)
                    eg = np.clip(le2[rt] - g * GCOL, 0, GCOL)
                    col = rt * N_G + g
                    empty = sg >= eg
                    full = (sg == 0) & (eg == GCOL)
                    c0 = eg.astype(np.float32)
                    c3 = sg.astype(np.float32)
                    c0[empty] = 0.0
                    c3[empty] = float(GCOL)
                    c0[full] = 0.0
                    c3[full] = 0.0
                    minc0[:, col] = c0
                    minc3[:, col] = c3
                for si, (sg_, lo, hi) in enumerate(segs[rt]):
                    scol = segcols[(rt, si)]
                    base = sg_ * GCOL + lo
                    L = hi - lo
                    maxs[:, scol] = np.clip(ls2[rt] - base, 0, L)
                    maxe[:, scol] = np.clip(le2[rt] - base, 0, L)
            in_maps.append({
                "lhsT": lhsT, "rhs": rhs, "nsq": nsq, "ones": ones,
                "minc0": minc0, "minc3": minc3, "maxs": maxs, "maxe": maxe,
            })
    return in_maps, core_rows, sq, fast


def kernel(inputs, targets):
    import concourse.bass_utils as bass_utils

    in_maps, core_rows, sq, fast = _host_prep(inputs, targets)

    key = bool(fast)
    if key not in _MODULES:
        _MODULES[key] = (_build_module_fast() if fast
                         else _build_module_fallback())
    nc = _MODULES[key]

    res = bass_utils.run_bass_kernel_spmd(
        nc, in_maps, core_ids=list(range(N_CORES)))

    d2ap = np.empty(N_TOTAL, dtype=np.float64)
    d2an = np.empty(N_TOTAL, dtype=np.float64)
    pos_cnt = np.empty(N_TOTAL, dtype=np.int64)
    neg_cnt = np.empty(N_TOTAL, dtype=np.int64)
    ptr = 0
    for c in range(N_CORES):
        out = res.results[c]["out"]          # [128, 32]
        a = out[:, 0::2].T.reshape(-1)       # max over positives of v
        mneg = out[:, 1::2].T.reshape(-1)    # max over negatives of v' = -min v
        rows, cnt = core_rows[c]
        d2ap[ptr:ptr + ROWS] = sq[rows] + a.astype(np.float64)
        d2an[ptr:ptr + ROWS] = sq[rows] - mneg.astype(np.float64)
        pos_cnt[ptr:ptr + ROWS] = cnt
        neg_cnt[ptr:ptr + ROWS] = HALF - cnt
        ptr += ROWS
    dist_ap = np.sqrt(np.clip(d2ap, 1e-12, None))
    dist_an = np.sqrt(np.clip(d2an, 1e-12, None))
    dist_ap = np.where(pos_cnt > 0, dist_ap, -np.inf)
    dist_an = np.where(neg_cnt > 0, dist_an, np.inf)
    diff = dist_ap - dist_an + MARGIN
    diff = np.where(np.isnan(diff), 0.0, diff)
    loss = np.maximum(diff, 0.0).mean()
    correct = int((dist_an >= dist_ap).sum())
    return (np.float32(loss), np.int32(correct))
